# revision 7
# baseline (speedup 1.0000x reference)
"""Trainium2 Bass kernel for equivariant multihead attention.

Math (per batch b, query point i, coset s1, channel c):
    logit[j,s2] = sum_g pairwise_g[b,i,j,s1,s2,g]*w_g[c,g]
                  + w_y[c,0]*y[b,j,s2,c] + (terms const over keys, dropped)
    att = exp(logit)*mask[b,j,s2];  att /= sum_{j,s2} att
    out = (y[b,i,s1,c] + sum_{j,s2} att*y[b,j,s2,c]) * mask[b,i,s1]  @ w_lin.T

The key-side factor exp(w_y[c,0]*y)*mask is FOLDED INTO THE LOGITS via a
second PE matmul accumulating into the same PSUM bank: 8 "y-feature" rows per
s2 (feature k carries w_y[k,0]*y[b,j,s2,k] + logmask[b,j,s2], with weight
delta(k,c)) add exactly w_y[c,0]*y[b,j,s2,c] + logmask to every (h,s2,c)
logit row.  E' = exp(L') then already includes mask and key factor, so per
(b,i) block only two j-reductions remain:
    den[(h,s2,c), q] = sum_j E'
    num[(h,s2,c), q] = sum_j E' * Y      (Y[(h,s2,c), j] = y[b,j,s2,c])
A final PE matmul sums the partials over s2; host does the residual add,
query mask and the c_in->c_out linear (tiny).

Per-block compute modes (tunable split to balance engines):
  "a_*" : logits as [r=(h,s2,c), (q,j)] (lhsT=bd const).  den by DVE
          segmented reduce ("a_dve") or act exp+accum ("a_act"); num by 4
          DVE scalar_tensor_tensor with the Y table.
  "b"   : logits TRANSPOSED as [j, (q,r)] (lhsT=pg data, rhs=bd).  den/num
          are then contraction-over-j matmuls: lhsT=e'_q (or m_q=e'*Y_T),
          rhs=ones -> [r,1] PSUM columns.  DVE only does one tensor_tensor
          multiply; PE absorbs both reductions.

pairwise_g is pre-transposed ON HOST to [(h,s2,g)=112 rows, (blk,q,j)] bf16
(no PE transposes / PSUM->SBUF copies on device; bf16 halves HBM traffic and
runs the PE at 1 cycle/row).

Sharding: query dim i is split 8 ways (16 i x 4 b = 64 blocks per core).
"""

import numpy as np
import ml_dtypes

import concourse.bacc as bacc
import concourse.tile as tile
from concourse import mybir
from concourse.bass_utils import run_bass_kernel_spmd

B, N, S, CIN, COUT, GDIM = 4, 128, 8, 8, 8, 7
NCORES = 8
ISHARD = N // NCORES          # 16 query points per core
NBLK = B * ISHARD             # 64 (b,i) blocks per core
NQ = 4                        # s1 pairs per block
BW = NQ * 128                 # 512 pg columns per block
NCOL = NBLK * NQ              # 256 partial columns per half
PGROWS = 2 * S * GDIM         # 112 rows: (h, s2, g)
EXTROWS = S * CIN             # 64 rows: (s2, k) y-features (mask folded in)
LOGMASK0 = -50.0              # logit offset for masked keys

# blocks covered by each of the 7 pg DMAs (earlier ones smaller for ramp)
SUPER_BLOCKS = (4, 6, 8, 10, 11, 12, 13)

# Mode split: blocks [0, NB_B) run mode "b" (contiguous so the psum->sbuf
# copy of their den/num columns is two contiguous ranges); of the rest,
# every ACT_EVERYth runs "a_act", others "a_dve".
NB_B = 36
ACT_EVERY = 5

F32 = mybir.dt.float32
BF16 = mybir.dt.bfloat16

# consts_bf16 column layout
BD0 = 0                        # bd [112, 128]
BDE0 = 128                     # bdext [64, 128]
Y0 = 256                       # Y tables [128, 128] x B  (rows (h,s2,c))
YT0 = Y0 + B * 128             # YT tables [128, 4*128] x B (rows j, x4 q)
EXT0 = YT0 + B * BW            # extrep [64, 512] x B
BDR0 = EXT0 + B * BW           # bdext tiled x4 [64, 512] (mode-b ext rhs)
ONES0 = BDR0 + BW              # ones column [128, 1]
CBW = ONES0 + 1
# consts_f32: sind [128, 16]
CFW = 16

_PROGRAM_CACHE = {}


def _mode(blk):
    if blk < NB_B:
        return "b"
    return "a_act" if (blk - NB_B) % ACT_EVERY == ACT_EVERY - 1 else "a_dve"


def _build_program(nblk=NBLK, loop_reps=1):
    """loop_reps>1 wraps the main loop in a hardware For_i that re-runs the
    full pass (including the input DMAs) on the same data -- used only for
    timing: wall(loop_reps=R) - wall(loop_reps=1) isolates device time from
    the ~100ms axon dispatch/transfer overhead."""
    nc = bacc.Bacc("TRN2", target_bir_lowering=False, debug=False,
                   num_devices=NCORES)

    pg_d = nc.dram_tensor("pg", (PGROWS, NBLK, NQ, 128), BF16,
                          kind="ExternalInput").ap()
    cb_d = nc.dram_tensor("cb", (128, CBW), BF16, kind="ExternalInput").ap()
    cf_d = nc.dram_tensor("cf", (128, CFW), F32, kind="ExternalInput").ap()
    out_s = nc.dram_tensor("out_s", (16, 2 * NCOL), F32,
                           kind="ExternalOutput").ap()

    supers = []
    blk0 = 0
    for nb in SUPER_BLOCKS:
        if blk0 >= nblk:
            break
        nb = min(nb, nblk - blk0)
        supers.append((blk0, blk0 + nb))
        blk0 += nb

    nb_b = min(NB_B, nblk)

    with tile.TileContext(nc) as tc:
        with (
            tc.tile_pool(name="consts", bufs=1) as consts,
            tc.tile_pool(name="epool", bufs=4) as epool,
            tc.tile_pool(name="mpool", bufs=4) as mpool,
            tc.tile_pool(name="psA", bufs=4, space="PSUM") as psA,
            tc.tile_pool(name="psB", bufs=1, space="PSUM") as psB,
            tc.tile_pool(name="psC", bufs=1, space="PSUM") as psC,
        ):
            pg_all = consts.tile([PGROWS, nblk, NQ, 128], BF16)
            cb = consts.tile([128, CBW], BF16)
            cf = consts.tile([128, CFW], F32)

            bd = cb[0:PGROWS, BD0:BD0 + 128]
            bdext = cb[0:EXTROWS, BDE0:BDE0 + 128]
            bdext_r = cb[0:EXTROWS, BDR0:BDR0 + BW]
            ones = cb[:, ONES0:ONES0 + 1]
            sind = cf[:, 0:16]

            buf = consts.tile([128, 2 * NCOL], F32)
            # den/num accumulation bank for mode-"b" blocks
            pnb = psB.tile([128, 2 * NCOL], F32, name="pnb") if nb_b > 0 else None

            NDUM = 8
            dummies = [consts.tile([128, 1], BF16, name=f"dum{i}")
                       for i in range(NDUM)]
            dum_idx = [0]
            s_sb = consts.tile([16, 2 * NCOL], F32)

            def block_a(blk, mode):
                b = blk // ISHARD
                y_b = cb[:, Y0 + b * 128:Y0 + (b + 1) * 128]
                ext_b = cb[0:EXTROWS, EXT0 + b * BW:EXT0 + (b + 1) * BW]
                pg_blk = pg_all[:, blk]

                l_ps = psA.tile([128, NQ, 128], F32, tag="lps")
                nc.tensor.matmul(l_ps, lhsT=bd, rhs=pg_blk,
                                 start=True, stop=False)
                nc.tensor.matmul(l_ps, lhsT=bdext, rhs=ext_b,
                                 start=False, stop=True)

                e_t = epool.tile([128, NQ, 128], BF16, tag="e")
                if mode == "a_act":
                    for q in range(NQ):
                        nc.scalar.activation(
                            e_t[:, q, :], l_ps[:, q, :],
                            mybir.ActivationFunctionType.Exp,
                            accum_out=buf[:, blk * NQ + q:blk * NQ + q + 1])
                else:
                    nc.scalar.activation(
                        e_t, l_ps, mybir.ActivationFunctionType.Exp)
                    nc.vector.tensor_reduce(
                        buf[:, blk * NQ:(blk + 1) * NQ], e_t,
                        axis=mybir.AxisListType.X, op=mybir.AluOpType.add)
                for q in range(NQ):
                    col = NCOL + blk * NQ + q
                    dum = dummies[dum_idx[0] % NDUM]
                    dum_idx[0] += 1
                    nc.vector.scalar_tensor_tensor(
                        dum.broadcast_to((128, 128)), e_t[:, q, :], 0.0, y_b,
                        op0=mybir.AluOpType.bypass, op1=mybir.AluOpType.mult,
                        accum_out=buf[:, col:col + 1])

            def block_b(blk):
                b = blk // ISHARD
                yt_b = cb[:, YT0 + b * BW:YT0 + (b + 1) * BW]
                ext1 = cb[0:EXTROWS, EXT0 + b * BW:EXT0 + b * BW + 128]
                pg3 = pg_all[:, blk]

                l_ps = psA.tile([128, NQ, 128], F32, tag="lps")
                nc.tensor.matmul(l_ps, lhsT=ext1, rhs=bdext_r,
                                 start=True, stop=False)
                for q in range(NQ):
                    nc.tensor.matmul(l_ps[:, q, :], lhsT=pg3[:, q, :], rhs=bd,
                                     start=False, stop=(q == NQ - 1))

                e_t = epool.tile([128, NQ, 128], BF16, tag="e")
                nc.scalar.activation(e_t, l_ps,
                                     mybir.ActivationFunctionType.Exp)
                m_t = mpool.tile([128, NQ, 128], BF16, tag="m")
                nc.vector.tensor_tensor(
                    m_t, e_t, yt_b.rearrange("p (q j) -> p q j", q=NQ),
                    op=mybir.AluOpType.mult)
                for q in range(NQ):
                    cd = blk * NQ + q
                    cn = NCOL + cd
                    nc.tensor.matmul(pnb[:, cd:cd + 1], lhsT=e_t[:, q, :],
                                     rhs=ones, start=True, stop=True)
                    nc.tensor.matmul(pnb[:, cn:cn + 1], lhsT=m_t[:, q, :],
                                     rhs=ones, start=True, stop=True)

            def main_pass():
                nc.sync.dma_start(cb, cb_d)
                nc.sync.dma_start(cf, cf_d)
                for (b0, b1) in supers:
                    nc.sync.dma_start(pg_all[:, b0:b1], pg_d[:, b0:b1])
                for (b0, b1) in supers:
                    for blk in range(b0, b1):
                        m = _mode(blk) if blk < nblk else "a_dve"
                        if m == "b" and blk < nb_b:
                            block_b(blk)
                        else:
                            block_a(blk, m)
                if nb_b > 0:
                    # move mode-"b" den/num partials psum -> buf
                    nc.vector.tensor_copy(buf[:, 0:nb_b * NQ],
                                          pnb[:, 0:nb_b * NQ])
                    nc.vector.tensor_copy(
                        buf[:, NCOL:NCOL + nb_b * NQ],
                        pnb[:, NCOL:NCOL + nb_b * NQ])

            if loop_reps > 1:
                with tc.For_i(0, loop_reps, 1,
                              hint_engines=(mybir.EngineType.PE,
                                            mybir.EngineType.Activation,
                                            mybir.EngineType.DVE,
                                            mybir.EngineType.SP)):
                    main_pass()
            else:
                main_pass()

            # sum the (h,s2,c) j-partials over s2 -> (h,c)
            s_ps = psC.tile([16, 2 * NCOL], F32)
            nc.tensor.matmul(s_ps, lhsT=sind, rhs=buf, start=True, stop=True)
            nc.scalar.copy(s_sb, s_ps)
            nc.sync.dma_start(out_s, s_sb)

    nc.compile()
    return nc


def _get_program(nblk=NBLK, loop_reps=1):
    key = ("nc", nblk, loop_reps, NB_B, ACT_EVERY)
    if key not in _PROGRAM_CACHE:
        _PROGRAM_CACHE[key] = _build_program(nblk, loop_reps)
    return _PROGRAM_CACHE[key]


def _host_prep(pairwise_g, coset_functions, mask, w_y, w_g):
    """Build the per-core input arrays."""
    y = coset_functions.astype(np.float32)          # (B, N, S, C) keys
    logmask = np.where(mask, 0.0, LOGMASK0).astype(np.float32)  # (B, j, s2)

    # bd [112, 128]: (h,s2,g) -> (h,s2,c) per-plane w_g
    bd = np.zeros((PGROWS, 128), np.float32)
    for pl in range(16):
        for g in range(GDIM):
            for c in range(CIN):
                bd[pl * GDIM + g, pl * CIN + c] = w_g[c, g]

    # bdext [64, 128]: y-feature row (s2,k) -> (h,s2,c) with weight d(k,c)
    bdext = np.zeros((EXTROWS, 128), np.float32)
    for h in range(2):
        for s2 in range(S):
            for c in range(CIN):
                bdext[s2 * CIN + c, (h * S + s2) * CIN + c] = 1.0

    # ext[b][(s2,k), j] = w_y[k,0]*y[b,j,s2,k] + logmask[b,j,s2]
    ext = (y.transpose(0, 2, 3, 1) * w_y[:, 0][None, None, :, None]
           + logmask.transpose(0, 2, 1)[:, :, None, :])  # (B, s2, k, j)
    ext = ext.reshape(B, EXTROWS, N)

    # Y[b][(h,s2,c), j] = y[b,j,s2,c];  YT[b][j, (h,s2,c)] = same, transposed
    ytab = np.tile(y.transpose(0, 2, 3, 1).reshape(B, S * CIN, N), (1, 2, 1))

    sind = np.zeros((128, 16), np.float32)
    for h in range(2):
        for s2 in range(S):
            for c in range(CIN):
                sind[h * 64 + s2 * CIN + c, h * CIN + c] = 1.0

    cb = np.zeros((128, CBW), np.float32)
    cb[0:PGROWS, BD0:BD0 + 128] = bd
    cb[0:EXTROWS, BDE0:BDE0 + 128] = bdext
    for b in range(B):
        cb[:, Y0 + b * 128:Y0 + (b + 1) * 128] = ytab[b]
        cb[:, YT0 + b * BW:YT0 + (b + 1) * BW] = np.tile(ytab[b].T, (1, NQ))
        cb[0:EXTROWS, EXT0 + b * BW:EXT0 + (b + 1) * BW] = np.tile(
            ext[b], (1, NQ))
    cb[0:EXTROWS, BDR0:BDR0 + BW] = np.tile(bdext, (1, NQ))
    cb[:, ONES0] = 1.0
    cb = cb.astype(ml_dtypes.bfloat16)

    cf = np.zeros((128, CFW), np.float32)
    cf[:, 0:16] = sind

    in_maps = []
    for k in range(NCORES):
        sl = slice(ISHARD * k, ISHARD * (k + 1))
        pgc = pairwise_g[:, sl]                      # (B, 16, j, s1, s2, g)
        pgc = pgc.reshape(B, ISHARD, N, NQ, 2, S, GDIM)
        # -> (b, i, h, s2, g, q, j)
        pgc = pgc.transpose(0, 1, 4, 5, 6, 3, 2)
        pgc = pgc.reshape(NBLK, PGROWS, NQ * N)
        pg = np.ascontiguousarray(pgc.transpose(1, 0, 2)).reshape(
            PGROWS, NBLK * BW).astype(ml_dtypes.bfloat16)
        in_maps.append({"pg": pg, "cb": cb, "cf": cf})
    return in_maps


def _host_finish(s_list, coset_functions, mask, w_lin):
    """Decode per-core (16, 512) outputs into the full result."""
    y = np.asarray(coset_functions, dtype=np.float32)
    maskf = np.asarray(mask).astype(np.float32)
    out = np.empty((B, N, S, COUT), np.float32)
    for k in range(NCORES):
        s = s_list[k]
        den = s[:, :NCOL].reshape(2, CIN, NBLK, NQ)
        num = s[:, NCOL:].reshape(2, CIN, NBLK, NQ)
        # (h, c, blk, q) -> (blk, s1 = 2q + h, c)
        den = den.transpose(2, 3, 0, 1).reshape(NBLK, S, CIN)
        num = num.transpose(2, 3, 0, 1).reshape(NBLK, S, CIN)
        sl = slice(ISHARD * k, ISHARD * (k + 1))
        y_q = y[:, sl].reshape(NBLK, S, CIN)
        m_q = maskf[:, sl].reshape(NBLK, S)
        res = (y_q + num / den) * m_q[..., None]
        res = res @ w_lin.T
        out[:, sl] = res.reshape(B, ISHARD, S, COUT)
    return out


def kernel(pairwise_g, coset_functions, mask, w_y, b_y, w_g, b_g, w_lin):
    pairwise_g = np.asarray(pairwise_g, dtype=np.float32)
    coset_functions = np.asarray(coset_functions, dtype=np.float32)
    mask = np.asarray(mask)
    w_y = np.asarray(w_y, dtype=np.float32)
    w_g = np.asarray(w_g, dtype=np.float32)
    w_lin = np.asarray(w_lin, dtype=np.float32)

    nc = _get_program()
    in_maps = _host_prep(pairwise_g, coset_functions, mask, w_y, w_g)
    res = run_bass_kernel_spmd(nc, in_maps, core_ids=list(range(NCORES)))
    s_list = [r["out_s"] for r in res.results]
    return _host_finish(s_list, coset_functions, mask, w_lin)
